# revision 6
# baseline (speedup 1.0000x reference)
"""ViT-style attention with decomposed relative position embeddings on 8 TRN2
NeuronCores. Data-parallel over batch (B=8 -> 1 image per core); weights and
the small rel-pos tables are replicated.

v2 schedule: the kernel is ScalarE-bound (96 exp instructions of [128,1024]
~= 117us), so everything is organized to (a) start the exp stream as early as
possible and (b) keep the PE busy inside the exp-rate-limited attention phase
with deferred filler matmuls (v-GEMM, k o-tiles, proj) using 2 spare PSUM
banks.

Per-core computation (one image, T=1024 tokens, C=768, 12 heads x 64):
  - qext layout [128part, 32a, 32b, 12h]: rows 0:64 q/8 (+bias), rows 64:96
    rel_h projections, rows 96:128 rel_w projections. This layout makes the
    rel-pos PSUM evacuations (the v1 bottleneck: strided 2.7-4.3us copies)
    contiguous-ish [32, 384] copies split across ScalarE/VectorE.
  - S^T = Kext^T . Qext per head and query-half (hf: 512 q's), exp on
    ScalarE in [128, 1024] chunks -> P^T (bf16)
  - PV: out^T[65, 512] = Vaug^T . P^T accumulated over 8 k-chunks; ones
    column of Vaug makes row 64 the softmax denominator.
  - normalization: denominator row DMA-reshaped [1,512]->[32,16], DVE
    reciprocal, DMA back + gpsimd partition-broadcast, one DVE multiply.
  - attention is processed in 24 slots = (hf=0: h0..h11, hf=1: h0..h11);
    PV for slot s issues during slot s+2 so the exp stream never waits.
  - proj for query-half 0 runs as filler during half-1 slots.

All matmuls bf16 (fp32 PSUM accumulation).
"""

import contextlib

import numpy as np
import ml_dtypes

BF16 = ml_dtypes.bfloat16

B, H, W, C = 8, 32, 32, 768
NH, HD, T = 12, 64, 1024
N_CORES = 8

_cache = {}


def _bf(a):
    return np.ascontiguousarray(np.asarray(a, dtype=np.float32)).astype(BF16)


def _f32(a):
    return np.ascontiguousarray(np.asarray(a, dtype=np.float32))


def _build_nc():
    if "nc" in _cache:
        return _cache["nc"]

    import concourse.mybir as mybir
    import concourse.tile as tile
    from concourse import bacc

    f32 = mybir.dt.float32
    bf16 = mybir.dt.bfloat16
    EXP = mybir.ActivationFunctionType.Exp

    nc = bacc.Bacc("TRN2", target_bir_lowering=False, debug=False)

    # ---- DRAM I/O ----
    xT_d = nc.dram_tensor("xT", [C, T], bf16, kind="ExternalInput")
    wqk_d = nc.dram_tensor("w_qk", [C, 2 * C], bf16, kind="ExternalInput")
    wv_d = nc.dram_tensor("w_v", [C, C], bf16, kind="ExternalInput")
    wp_d = nc.dram_tensor("w_p", [C, C], bf16, kind="ExternalInput")
    bqk_d = nc.dram_tensor("b_qk", [128, 12], f32, kind="ExternalInput")
    bv_d = nc.dram_tensor("b_v", [1, C], f32, kind="ExternalInput")
    bp_d = nc.dram_tensor("b_p", [1, C], f32, kind="ExternalInput")
    relt_d = nc.dram_tensor("relt", [64, 2048], bf16, kind="ExternalInput")
    oneh_d = nc.dram_tensor("onehot", [64, 32, 32], bf16, kind="ExternalInput")
    out_d = nc.dram_tensor("out", [T, C], f32, kind="ExternalOutput")

    with tile.TileContext(nc) as tc:
        es = contextlib.ExitStack()
        cp = es.enter_context(tc.tile_pool(name="const", bufs=1))

        # ---- persistent SBUF tensors ----
        xT = cp.tile([128, 6, T], bf16, tag="xT")
        wqk = cp.tile([128, 6, 2 * C], bf16, tag="wqk")
        wv = cp.tile([128, 6, C], bf16, tag="wv")
        wpr = cp.tile([128, 6, C], bf16, tag="wpr")
        bqk = cp.tile([128, 12], f32, tag="bqk")
        bv_row = cp.tile([1, C], f32, tag="bv_row")
        bp_row = cp.tile([1, C], f32, tag="bp_row")
        bv_bc = cp.tile([128, NH, HD], f32, tag="bv_bc")
        bp_bc = cp.tile([128, C], f32, tag="bp_bc")
        relt = cp.tile([64, 2048], bf16, tag="relt")
        # qext: [part, a(row), b(col), head]; rows 0:64 = q features,
        # 64:96 rel_h (kh j), 96:128 rel_w (kw j)
        qext = cp.tile([128, 32, 32, NH], bf16, tag="qext")
        # kext: [part, head, a, b]; rows 0:64 = k features, 64:128 onehot
        kext = cp.tile([128, NH, 32, 32], bf16, tag="kext")
        vaug = cp.tile([128, 8, NH, 65], bf16, tag="vaug")
        yall = cp.tile([128, 6, T], bf16, tag="yall")

        # P buffers (exp output) - 3 bufs to support PV deferral of 2 slots
        pp = es.enter_context(tc.tile_pool(name="pbuf", bufs=3))
        # normalization scratch
        np_pool = es.enter_context(tc.tile_pool(name="norm", bufs=2))
        zp = es.enter_context(tc.tile_pool(name="zout", bufs=2))

        # ---- input DMAs (interleaved so matmul (ot=0, c=0) starts early) ----
        nc.sync.dma_start(bqk[:], bqk_d[:])
        for c in range(6):
            nc.sync.dma_start(xT[:, c, :], xT_d[c * 128:(c + 1) * 128, :])
            nc.sync.dma_start(wqk[:, c, :], wqk_d[c * 128:(c + 1) * 128, :])
        for c in range(6):
            nc.sync.dma_start(wv[:, c, :], wv_d[c * 128:(c + 1) * 128, :])
        nc.sync.dma_start(relt[:], relt_d[:])
        for h in range(NH):
            nc.sync.dma_start(kext[64:128, h, :, :], oneh_d[:])
        nc.sync.dma_start(bv_row[:], bv_d[:])
        nc.sync.dma_start(bp_row[:], bp_d[:])
        for c in range(6):
            nc.sync.dma_start(wpr[:, c, :], wp_d[c * 128:(c + 1) * 128, :])
        nc.gpsimd.partition_broadcast(bv_bc[:], bv_row[:])
        nc.gpsimd.partition_broadcast(bp_bc[:], bp_row[:])
        nc.gpsimd.memset(vaug[:, :, :, 64:65], 1.0)

        # ======== phase A: q o-tiles (heads' q), then k pairs 0-1 ========
        def qk_otile(ps, ot, tag="qk"):
            acc = ps.tile([128, 32, 32], f32, tag=tag, name=f"qk_{ot}")
            for c in range(6):
                for hf in range(2):
                    nc.tensor.matmul(
                        acc[:, hf * 16:(hf + 1) * 16, :],
                        wqk[:, c, ot * 128:(ot + 1) * 128],
                        xT[:, c, hf * 512:(hf + 1) * 512],
                        start=(c == 0), stop=(c == 5),
                    )
            is_q = ot < 6
            hp = ot if is_q else ot - 6
            for half in range(2):
                head = 2 * hp + half
                src = acc[64 * half:64 * (half + 1), :, :]
                bias = bqk[64 * half:64 * (half + 1), ot:ot + 1]
                if is_q:
                    dst = qext[0:64, :, :, head]
                else:
                    dst = kext[0:64, head, :, :]
                nc.vector.tensor_scalar_add(dst, src, bias)

        with tc.tile_pool(name="ps_qk", bufs=2, space="PSUM") as ps_qk:
            for ot in (0, 1, 2, 3, 4, 5, 6, 7):
                qk_otile(ps_qk, ot)

        # pools for the rest of the kernel; pools release LIFO, so ps_rel
        # (released mid-kernel, its banks reused by ps_pv) is opened last.
        ps_aux = es.enter_context(
            tc.tile_pool(name="ps_aux", bufs=1, space="PSUM"))
        ps_s = es.enter_context(
            tc.tile_pool(name="ps_s", bufs=2, space="PSUM"))
        rel_es = contextlib.ExitStack()
        ps_rel = rel_es.enter_context(
            tc.tile_pool(name="ps_rel", bufs=2, space="PSUM"))

        def v_tile(tt):
            accv = ps_aux.tile([128, 32, 32], f32, tag="aux",
                               name=f"v_{tt}").rearrange(
                                   "p a b -> p (a b)")[:, 0:768].rearrange(
                                   "p (h d) -> p h d", h=NH)
            for c in range(6):
                nc.tensor.matmul(
                    accv[:, 0:8, :],
                    xT[:, c, tt * 128:(tt + 1) * 128],
                    wv[:, c, 0:512],
                    start=(c == 0), stop=(c == 5),
                )
                nc.tensor.matmul(
                    accv[:, 8:12, :],
                    xT[:, c, tt * 128:(tt + 1) * 128],
                    wv[:, c, 512:768],
                    start=(c == 0), stop=(c == 5),
                )
            nc.vector.tensor_add(vaug[:, tt, :, 0:64], accv[:], bv_bc[:])

        def k_pair(hp):
            qk_otile(ps_aux2, 6 + hp)

        # ---- rel block: 32 iterations (one image row/col x), interleaved
        # with v tiles 0-5 so the PE stays busy while DVE/ACT evacuate.
        # rel_h (x=a): out[j, (b,h)] at partitions 64:96
        # rel_w (x=b): out[j, (a,h)] at partitions 96:128, both N=384,
        # col-tiled so they run concurrently.
        def rel_iter(x):
            accr = ps_rel.tile([128, 32, NH], f32, tag="rel")
            nc.tensor.matmul(
                accr[64:96, :, :],
                relt[0:64, x * 32:x * 32 + 32],
                qext[0:64, x, :, :],
                start=True, stop=True, tile_position=(0, 64),
            )
            nc.tensor.matmul(
                accr[96:128, :, :],
                relt[0:64, 1024 + x * 32:1024 + x * 32 + 32],
                qext[0:64, :, x, :],
                start=True, stop=True, tile_position=(0, 96),
            )
            # evacuations: dst_h contiguous -> ScalarE; dst_w strided -> DVE
            nc.scalar.copy(qext[64:96, x, :, :], accr[64:96, :, :])
            nc.vector.tensor_copy(qext[96:128, :, x, :], accr[96:128, :, :])

        for x in range(32):
            rel_iter(x)
            if x % 5 == 4:
                v_tile(x // 5)  # v0..v5

        # ================= attention slots =================
        # slot s = (hf, h); S+exp immediately, PV deferred by DEFER slots.
        DEFER = 2
        slots = [(hf, h) for hf in range(2) for h in range(12)]
        p_bufs = {}
        acc_pv = {}

        def emit_S(si):
            hf, h = slots[si]
            p_t = pp.tile([128, 8, 512], bf16, tag="P", name=f"p_{si}")
            p_bufs[si] = p_t
            for jp in range(4):
                accs = ps_s.tile([128, 2, 512], f32, tag="S")
                for half in range(2):
                    kt = 2 * jp + half
                    nc.tensor.matmul(
                        accs[:, half, :],
                        kext[:, h, kt * 4:(kt + 1) * 4, :],
                        qext[:, hf * 16:(hf + 1) * 16, :, h],
                        start=True, stop=True,
                    )
                nc.scalar.activation(p_t[:, 2 * jp:2 * jp + 2, :], accs[:], EXP)

        def emit_PV(si):
            hf, h = slots[si]
            p_t = p_bufs.pop(si)
            accp = ps_pv.tile([65, 512], f32, tag="PV")
            acc_pv[si] = accp
            for kt in range(8):
                nc.tensor.matmul(
                    accp[:],
                    vaug[:, kt, h, :],
                    p_t[:, kt, :],
                    start=(kt == 0), stop=(kt == 7),
                )

        def emit_norm(si):
            hf, h = slots[si]
            accp = acc_pv.pop(si)
            d_sq = np_pool.tile([32, 16], f32, tag="dsq")
            d_row = np_pool.tile([1, 512], f32, tag="drow")
            r_row = np_pool.tile([1, 512], f32, tag="rrow")
            r_bc = np_pool.tile([64, 512], f32, tag="rbc")
            nc.vector.tensor_copy(d_row[:], accp[64:65, :])
            nc.sync.dma_start(d_sq[:], d_row[:])
            nc.vector.reciprocal(d_sq[:], d_sq[:])
            nc.sync.dma_start(r_row[:], d_sq[:])
            nc.gpsimd.partition_broadcast(r_bc[:], r_row[:])
            nc.vector.tensor_mul(
                yall[64 * (h % 2):64 * (h % 2 + 1), h // 2,
                     hf * 512:(hf + 1) * 512],
                accp[0:64, :], r_bc[:])

        def emit_proj(hf, tt):
            # tt in 0..3 within this half; global token tile = hf*4 + tt
            g = hf * 4 + tt
            accz = ps_aux.tile([128, C], f32, tag="aux")
            for p in range(6):
                nc.tensor.matmul(
                    accz[:, 0:512],
                    yall[:, p, g * 128:(g + 1) * 128],
                    wpr[:, p, 0:512],
                    start=(p == 0), stop=(p == 5),
                )
                nc.tensor.matmul(
                    accz[:, 512:768],
                    yall[:, p, g * 128:(g + 1) * 128],
                    wpr[:, p, 512:768],
                    start=(p == 0), stop=(p == 5),
                )
            z_t = zp.tile([128, C], f32, tag="Zt")
            nc.vector.tensor_add(z_t[:], accz[:], bp_bc[:])
            nc.sync.dma_start(out_d[g * 128:(g + 1) * 128, :], z_t[:])

        # filler schedule: slot index -> list of thunks emitted after S
        fillers = {
            0: [lambda: v_tile(6)],
            1: [lambda: v_tile(7)],
            2: [lambda: qk_otile(ps_aux, 8, 'aux')],
            4: [lambda: qk_otile(ps_aux, 9, 'aux')],
            6: [lambda: qk_otile(ps_aux, 10, 'aux')],
            8: [lambda: qk_otile(ps_aux, 11, 'aux')],
            14: [lambda: emit_proj(0, 0)],
            16: [lambda: emit_proj(0, 1)],
            18: [lambda: emit_proj(0, 2)],
            20: [lambda: emit_proj(0, 3)],
        }

        emit_S(0)
        emit_S(1)
        for f in fillers.get(0, []):
            f()
        for f in fillers.get(1, []):
            f()
        rel_es.close()
        ps_pv = es.enter_context(
            tc.tile_pool(name="ps_pv", bufs=2, space="PSUM"))
        for si in range(2, 24):
            emit_S(si)
            for f in fillers.get(si, []):
                f()
            emit_PV(si - DEFER)
            emit_norm(si - DEFER)
        for si in range(24 - DEFER, 24):
            emit_PV(si)
            emit_norm(si)
        for tt in range(4):
            emit_proj(1, tt)

        es.close()

    nc.compile()
    _cache["nc"] = nc
    return nc


def _host_prep(x, w_qkv, b_qkv, w_proj, b_proj, rel_pos_h, rel_pos_w):
    scale = HD ** -0.5
    w_qkv = _f32(w_qkv)
    b_qkv = _f32(b_qkv)

    w_qk = w_qkv[:, : 2 * C].copy()
    w_qk[:, :C] *= scale
    b_qk_flat = b_qkv[: 2 * C].copy()
    b_qk_flat[:C] *= scale
    b_qk = np.ascontiguousarray(b_qk_flat.reshape(12, 128).T)  # [128, 12]

    # relt [64, 2048]: cols tbl*1024 + x*32 + j -> 8*rel_pos[x - j + 31, :]
    idx = np.arange(32)[:, None] - np.arange(32)[None, :] + 31  # [x, j]
    relt = np.concatenate(
        [
            (8.0 * _f32(rel_pos_h))[idx].transpose(2, 0, 1).reshape(64, 1024),
            (8.0 * _f32(rel_pos_w))[idx].transpose(2, 0, 1).reshape(64, 1024),
        ],
        axis=1,
    )

    k = np.arange(T)
    onehot = np.zeros((64, T), np.float32)
    onehot[k // 32, k] = 1.0
    onehot[32 + k % 32, k] = 1.0

    shared = {
        "w_qk": _bf(w_qk),
        "w_v": _bf(w_qkv[:, 2 * C:]),
        "w_p": _bf(w_proj),
        "b_qk": _f32(b_qk),
        "b_v": _f32(b_qkv[2 * C:])[None, :],
        "b_p": _f32(b_proj)[None, :],
        "relt": _bf(relt),
        "onehot": _bf(onehot).reshape(64, 32, 32),
    }
    x = _f32(x)
    in_maps = []
    for i in range(N_CORES):
        m = dict(shared)
        m["xT"] = _bf(x[i].reshape(T, C).T)
        in_maps.append(m)
    return in_maps


def kernel(x, w_qkv, b_qkv, w_proj, b_proj, rel_pos_h, rel_pos_w):
    from concourse.bass_utils import run_bass_kernel_spmd

    nc = _build_nc()
    in_maps = _host_prep(x, w_qkv, b_qkv, w_proj, b_proj, rel_pos_h, rel_pos_w)
    res = run_bass_kernel_spmd(nc, in_maps, core_ids=list(range(N_CORES)))
    out = np.stack([_f32(res.results[i]["out"]) for i in range(N_CORES)])
    return out.reshape(B, H, W, C)


# revision 9
# speedup vs baseline: 1.4788x; 1.4788x over previous
"""ViT-style attention with decomposed relative position embeddings on 8 TRN2
NeuronCores. Data-parallel over batch (B=8 -> 1 image per core); weights and
the small rel-pos tables are replicated.

v3: the kernel is ScalarE-bound (96 exp instructions ~1.15us each), so the
schedule (a) starts the exp stream as early as possible and (b) fills the PE
inside the exp-rate-limited attention phase with deferred matmuls (v-GEMM,
k o-tiles, proj) through 2 spare PSUM banks.

Measured AP costs drive the layouts (microbench): a matmul rhs must be
contiguous (stride-12 rhs = 2.8x slower), so qext/kext keep the head-major
[128, 12h, 32a, 32b] layout and the S free dim is (a, b)-contiguous.
rel_h evacuates directly ([32,12,32] dst stride (1024,1) = 555ns). rel_w
needs an a<->b transpose somewhere; cheapest is contiguous staging slabs
(552ns each) + 12 per-head gather-transposes (2.6us DVE / 4.8us gpsimd)
which pipeline per-head into the attention phase (S of head h only needs
head h's gather).

Per-core computation (one image, T=1024 tokens, C=768, 12 heads x 64):
  - S^T = Kext^T . Qext per head and query-half (hf: 512 q's); contraction
    128 = 64 qk dims + 32 rel_h rows + 32 rel_w rows vs onehot rows of Kext,
    so the decomposed rel-pos additions ride in the S matmul for free.
  - exp on ScalarE in [128, 2, 512] chunks -> P^T (bf16)
  - PV: out^T[65, 512] = Vaug^T . P^T accumulated over 8 k-chunks; ones
    column of Vaug makes row 64 the softmax denominator.
  - normalization: denominator row DVE-copied + DMA-reshaped [1,512]->
    [32,16], DVE reciprocal, DMA back + gpsimd partition-broadcast, one DVE
    multiply (all off the Scalar/Tensor critical path).
  - attention in 24 slots = (hf=0: h0..h11, hf=1: h0..h11); PV of slot s
    issues during slot s+2 so the exp stream never waits on PSUM.
  - proj for query-half 0 runs as filler during half-1 slots.

All matmuls bf16 (fp32 PSUM accumulation).
"""

import contextlib

import numpy as np
import ml_dtypes

BF16 = ml_dtypes.bfloat16

B, H, W, C = 8, 32, 32, 768
NH, HD, T = 12, 64, 1024
N_CORES = 8

_cache = {}


def _bf(a):
    return np.ascontiguousarray(np.asarray(a, dtype=np.float32)).astype(BF16)


def _f32(a):
    return np.ascontiguousarray(np.asarray(a, dtype=np.float32))


def _build_nc():
    if "nc" in _cache:
        return _cache["nc"]

    import concourse.mybir as mybir
    import concourse.tile as tile
    from concourse import bacc

    f32 = mybir.dt.float32
    bf16 = mybir.dt.bfloat16
    EXP = mybir.ActivationFunctionType.Exp

    nc = bacc.Bacc("TRN2", target_bir_lowering=False, debug=False)

    # ---- DRAM I/O ----
    xT_d = nc.dram_tensor("xT", [C, T], bf16, kind="ExternalInput")
    wqk_d = nc.dram_tensor("w_qk", [C, 2 * C], bf16, kind="ExternalInput")
    wv_d = nc.dram_tensor("w_v", [C, C], bf16, kind="ExternalInput")
    wp_d = nc.dram_tensor("w_p", [C, C], bf16, kind="ExternalInput")
    bqk_d = nc.dram_tensor("b_qk", [128, 12], f32, kind="ExternalInput")
    bv_d = nc.dram_tensor("b_v", [1, C], f32, kind="ExternalInput")
    bp_d = nc.dram_tensor("b_p", [1, C], f32, kind="ExternalInput")
    relt_d = nc.dram_tensor("relt", [64, 2048], bf16, kind="ExternalInput")
    oneh_d = nc.dram_tensor("onehot", [64, 32, 32], bf16, kind="ExternalInput")
    out_d = nc.dram_tensor("out", [T, C], f32, kind="ExternalOutput")

    with tile.TileContext(nc) as tc:
        es = contextlib.ExitStack()
        cp = es.enter_context(tc.tile_pool(name="const", bufs=1))

        # ---- persistent SBUF tensors ----
        xT = cp.tile([128, 6, T], bf16, tag="xT")
        wqk = cp.tile([128, 6, 2 * C], bf16, tag="wqk")
        wv = cp.tile([128, 6, C], bf16, tag="wv")
        wpr = cp.tile([128, 6, C], bf16, tag="wpr")
        bqk = cp.tile([128, 12], f32, tag="bqk")
        bv_row = cp.tile([1, C], f32, tag="bv_row")
        bp_row = cp.tile([1, C], f32, tag="bp_row")
        bv_bc = cp.tile([128, NH, HD], f32, tag="bv_bc")
        bp_bc = cp.tile([128, C], f32, tag="bp_bc")
        relt = cp.tile([64, 2048], bf16, tag="relt")
        # qext: [part, head, a(row), b(col)]; rows 0:64 = q/8 (+bias),
        # rows 64:96 rel_h (kh j), rows 96:128 rel_w (kw j)
        qext = cp.tile([128, NH, 32, 32], bf16, tag="qext")
        # kext: [part, head, a, b]; rows 0:64 = k (+bias), 64:128 onehot
        kext = cp.tile([128, NH, 32, 32], bf16, tag="kext")
        vaug = cp.tile([128, 8, NH, 65], bf16, tag="vaug")
        yall = cp.tile([128, 6, T], bf16, tag="yall")
        # rel_w staging: [j, b, head, a] (per-b slabs land contiguous)
        stg_w = cp.tile([32, 32, NH, 32], bf16, tag="stg_w")

        # P buffers (exp output) - 3 bufs to support PV deferral of 2 slots
        pp = es.enter_context(tc.tile_pool(name="pbuf", bufs=3))
        np_pool = es.enter_context(tc.tile_pool(name="norm", bufs=2))
        zp = es.enter_context(tc.tile_pool(name="zout", bufs=2))

        # ---- input DMAs (interleaved so matmul (ot=0, c=0) starts early) ----
        nc.sync.dma_start(bqk[:], bqk_d[:])
        for c in range(6):
            nc.sync.dma_start(xT[:, c, :], xT_d[c * 128:(c + 1) * 128, :])
            nc.sync.dma_start(wqk[:, c, :], wqk_d[c * 128:(c + 1) * 128, :])
        for c in range(6):
            nc.sync.dma_start(wv[:, c, :], wv_d[c * 128:(c + 1) * 128, :])
        nc.sync.dma_start(relt[:], relt_d[:])
        for h in range(NH):
            nc.sync.dma_start(kext[64:128, h, :, :], oneh_d[:])
        nc.sync.dma_start(bv_row[:], bv_d[:])
        nc.sync.dma_start(bp_row[:], bp_d[:])
        for c in range(6):
            nc.sync.dma_start(wpr[:, c, :], wp_d[c * 128:(c + 1) * 128, :])
        nc.gpsimd.partition_broadcast(bv_bc[:], bv_row[:])
        nc.gpsimd.partition_broadcast(bp_bc[:], bp_row[:])
        nc.gpsimd.memset(vaug[:, :, :, 64:65], 1.0)

        # ======== phase A: q o-tiles, then k pairs 0-1 ========
        def qk_otile(ps, ot, tag="qk"):
            acc = ps.tile([128, 32, 32], f32, tag=tag, name=f"qk_{ot}")
            for c in range(6):
                for hf in range(2):
                    nc.tensor.matmul(
                        acc[:, hf * 16:(hf + 1) * 16, :],
                        wqk[:, c, ot * 128:(ot + 1) * 128],
                        xT[:, c, hf * 512:(hf + 1) * 512],
                        start=(c == 0), stop=(c == 5),
                    )
            is_q = ot < 6
            hp = ot if is_q else ot - 6
            for half in range(2):
                head = 2 * hp + half
                src = acc[64 * half:64 * (half + 1), :, :]
                bias = bqk[64 * half:64 * (half + 1), ot:ot + 1]
                if is_q:
                    dst = qext[0:64, head, :, :]
                else:
                    dst = kext[0:64, head, :, :]
                nc.vector.tensor_scalar_add(dst, src, bias)

        with tc.tile_pool(name="ps_qk", bufs=2, space="PSUM") as ps_qk:
            for ot in (0, 1, 2, 3, 4, 5, 6, 7):
                qk_otile(ps_qk, ot)

        # pools (released LIFO; ps_rel released mid-kernel -> banks to ps_pv)
        ps_aux = es.enter_context(
            tc.tile_pool(name="ps_aux", bufs=1, space="PSUM"))
        ps_s = es.enter_context(
            tc.tile_pool(name="ps_s", bufs=2, space="PSUM"))
        rel_es = contextlib.ExitStack()
        ps_rel = rel_es.enter_context(
            tc.tile_pool(name="ps_rel", bufs=2, space="PSUM"))

        def v_tile(tt):
            accv = ps_aux.tile([128, 32, 32], f32, tag="aux",
                               name=f"v_{tt}").rearrange(
                                   "p a b -> p (a b)")[:, 0:768].rearrange(
                                   "p (h d) -> p h d", h=NH)
            for c in range(6):
                nc.tensor.matmul(
                    accv[:, 0:8, :],
                    xT[:, c, tt * 128:(tt + 1) * 128],
                    wv[:, c, 0:512],
                    start=(c == 0), stop=(c == 5),
                )
                nc.tensor.matmul(
                    accv[:, 8:12, :],
                    xT[:, c, tt * 128:(tt + 1) * 128],
                    wv[:, c, 512:768],
                    start=(c == 0), stop=(c == 5),
                )
            nc.vector.tensor_add(vaug[:, tt, :, 0:64], accv[:], bv_bc[:])

        # ---- rel block: 32 iterations (one image row/col x).
        # rel_h (x=a): out [j, (h, b)] partitions 64:96, evac direct.
        # rel_w (x=b): out [j, (h, a)] partitions 96:128, evac to staging;
        # per-head gather-transposes move staging -> qext rows 96:128.
        def rel_iter(x):
            accr = ps_rel.tile([128, NH, 32], f32, tag="rel")
            nc.tensor.matmul(
                accr[64:96, :, :],
                relt[0:64, x * 32:x * 32 + 32],
                qext[0:64, :, x, :],
                start=True, stop=True, tile_position=(0, 64),
            )
            nc.tensor.matmul(
                accr[96:128, :, :],
                relt[0:64, 1024 + x * 32:1024 + x * 32 + 32],
                qext[0:64, :, :, x],
                start=True, stop=True, tile_position=(0, 96),
            )
            if x % 2 == 0:
                nc.scalar.copy(qext[64:96, :, x, :], accr[64:96, :, :])
            else:
                nc.vector.tensor_copy(qext[64:96, :, x, :], accr[64:96, :, :])
            nc.vector.tensor_copy(stg_w[:, x, :, :], accr[96:128, :, :])

        def gather_w(h, eng):
            dst = qext[96:128, h, :, :].rearrange("p a b -> p b a")
            src = stg_w[:, :, h, :]
            if eng == "v":
                nc.vector.tensor_copy(dst, src)
            else:
                nc.gpsimd.tensor_copy(dst, src)

        for x in range(32):
            rel_iter(x)
            if x % 5 == 4:
                v_tile(x // 5)  # v0..v5 keep the PE busy during rel evacs
        # gathers for the first heads before attention starts
        gather_w(0, "v")
        gather_w(1, "v")
        gather_w(2, "v")

        # ================= attention slots =================
        DEFER = 2
        slots = [(hf, h) for hf in range(2) for h in range(12)]
        p_bufs = {}
        acc_pv = {}

        def emit_S(si):
            hf, h = slots[si]
            p_t = pp.tile([128, 8, 512], bf16, tag="P", name=f"p_{si}")
            p_bufs[si] = p_t
            for jp in range(4):
                accs = ps_s.tile([128, 2, 512], f32, tag="S")
                for half in range(2):
                    kt = 2 * jp + half
                    nc.tensor.matmul(
                        accs[:, half, :],
                        kext[:, h, kt * 4:(kt + 1) * 4, :],
                        qext[:, h, hf * 16:(hf + 1) * 16, :],
                        start=True, stop=True,
                    )
                nc.scalar.activation(p_t[:, 2 * jp:2 * jp + 2, :], accs[:], EXP)

        def emit_PV(si):
            hf, h = slots[si]
            p_t = p_bufs.pop(si)
            accp = ps_pv.tile([65, 512], f32, tag="PV")
            acc_pv[si] = accp
            for kt in range(8):
                nc.tensor.matmul(
                    accp[:],
                    vaug[:, kt, h, :],
                    p_t[:, kt, :],
                    start=(kt == 0), stop=(kt == 7),
                )

        def emit_norm(si):
            hf, h = slots[si]
            accp = acc_pv.pop(si)
            d_sq = np_pool.tile([32, 16], f32, tag="dsq")
            d_row = np_pool.tile([1, 512], f32, tag="drow")
            r_row = np_pool.tile([1, 512], f32, tag="rrow")
            r_bc = np_pool.tile([64, 512], f32, tag="rbc")
            nc.vector.tensor_copy(d_row[:], accp[64:65, :])
            nc.sync.dma_start(d_sq[:], d_row[:])
            nc.vector.reciprocal(d_sq[:], d_sq[:])
            nc.sync.dma_start(r_row[:], d_sq[:])
            nc.gpsimd.partition_broadcast(r_bc[:], r_row[:])
            nc.vector.tensor_mul(
                yall[64 * (h % 2):64 * (h % 2 + 1), h // 2,
                     hf * 512:(hf + 1) * 512],
                accp[0:64, :], r_bc[:])

        def emit_proj(hf, tt):
            g = hf * 4 + tt
            accz = ps_aux.tile([128, C], f32, tag="aux", name=f"z_{g}")
            for p in range(6):
                nc.tensor.matmul(
                    accz[:, 0:512],
                    yall[:, p, g * 128:(g + 1) * 128],
                    wpr[:, p, 0:512],
                    start=(p == 0), stop=(p == 5),
                )
                nc.tensor.matmul(
                    accz[:, 512:768],
                    yall[:, p, g * 128:(g + 1) * 128],
                    wpr[:, p, 512:768],
                    start=(p == 0), stop=(p == 5),
                )
            z_t = zp.tile([128, C], f32, tag="Zt")
            nc.vector.tensor_add(z_t[:], accz[:], bp_bc[:])
            nc.sync.dma_start(out_d[g * 128:(g + 1) * 128, :], z_t[:])

        # filler schedule: slot -> thunks emitted after that slot's S.
        # Ordering constraints: k o-tile 6+p must precede S of head 2p
        # (slot 2p); gather_w(h) must precede S of head h (slot h); all
        # v tiles must precede the first PV (slot DEFER).
        fillers = {
            0: [lambda: v_tile(6), lambda: gather_w(3, "g")],
            1: [lambda: v_tile(7), lambda: gather_w(4, "v")],
            2: [lambda: qk_otile(ps_aux, 8, "aux"), lambda: gather_w(5, "g")],
            3: [lambda: qk_otile(ps_aux, 9, "aux"), lambda: gather_w(6, "v")],
            4: [lambda: qk_otile(ps_aux, 10, "aux"), lambda: gather_w(7, "g")],
            5: [lambda: qk_otile(ps_aux, 11, "aux"), lambda: gather_w(8, "v")],
            6: [lambda: gather_w(9, "g")],
            7: [lambda: gather_w(10, "v")],
            8: [lambda: gather_w(11, "g")],
            14: [lambda: emit_proj(0, 0)],
            16: [lambda: emit_proj(0, 1)],
            18: [lambda: emit_proj(0, 2)],
            20: [lambda: emit_proj(0, 3)],
        }

        emit_S(0)
        emit_S(1)
        for f in fillers.get(0, []):
            f()
        for f in fillers.get(1, []):
            f()
        rel_es.close()
        ps_pv = es.enter_context(
            tc.tile_pool(name="ps_pv", bufs=2, space="PSUM"))
        for si in range(2, 24):
            emit_S(si)
            for f in fillers.get(si, []):
                f()
            emit_PV(si - DEFER)
            emit_norm(si - DEFER)
        for si in range(24 - DEFER, 24):
            emit_PV(si)
            emit_norm(si)
        for tt in range(4):
            emit_proj(1, tt)

        es.close()

    nc.compile()
    _cache["nc"] = nc
    return nc


def _host_prep(x, w_qkv, b_qkv, w_proj, b_proj, rel_pos_h, rel_pos_w):
    scale = HD ** -0.5
    w_qkv = _f32(w_qkv)
    b_qkv = _f32(b_qkv)

    w_qk = w_qkv[:, : 2 * C].copy()
    w_qk[:, :C] *= scale
    b_qk_flat = b_qkv[: 2 * C].copy()
    b_qk_flat[:C] *= scale
    b_qk = np.ascontiguousarray(b_qk_flat.reshape(12, 128).T)  # [128, 12]

    # relt [64, 2048]: cols tbl*1024 + x*32 + j -> 8*rel_pos[x - j + 31, :]
    idx = np.arange(32)[:, None] - np.arange(32)[None, :] + 31  # [x, j]
    relt = np.concatenate(
        [
            (8.0 * _f32(rel_pos_h))[idx].transpose(2, 0, 1).reshape(64, 1024),
            (8.0 * _f32(rel_pos_w))[idx].transpose(2, 0, 1).reshape(64, 1024),
        ],
        axis=1,
    )

    k = np.arange(T)
    onehot = np.zeros((64, T), np.float32)
    onehot[k // 32, k] = 1.0
    onehot[32 + k % 32, k] = 1.0

    shared = {
        "w_qk": _bf(w_qk),
        "w_v": _bf(w_qkv[:, 2 * C:]),
        "w_p": _bf(w_proj),
        "b_qk": _f32(b_qk),
        "b_v": _f32(b_qkv[2 * C:])[None, :],
        "b_p": _f32(b_proj)[None, :],
        "relt": _bf(relt),
        "onehot": _bf(onehot).reshape(64, 32, 32),
    }
    x = _f32(x)
    in_maps = []
    for i in range(N_CORES):
        m = dict(shared)
        m["xT"] = _bf(x[i].reshape(T, C).T)
        in_maps.append(m)
    return in_maps


def kernel(x, w_qkv, b_qkv, w_proj, b_proj, rel_pos_h, rel_pos_w):
    from concourse.bass_utils import run_bass_kernel_spmd

    nc = _build_nc()
    in_maps = _host_prep(x, w_qkv, b_qkv, w_proj, b_proj, rel_pos_h, rel_pos_w)
    res = run_bass_kernel_spmd(nc, in_maps, core_ids=list(range(N_CORES)))
    out = np.stack([_f32(res.results[i]["out"]) for i in range(N_CORES)])
    return out.reshape(B, H, W, C)
